# revision 35
# baseline (speedup 1.0000x reference)
"""Bass/Trainium2 kernel for nn_CPdecomposition (CP-decomposition grid-sample MLP head).

Math (see reference):
  out[n, o] = sigmoid( sum_{comp<16} prod_{cin<6} val[c, n, cin] ),  c = comp*8 + o
  val[c, n, cin] = bilinear sample of plane[c] at (fixed W coord per cin, H coord = 5*x[n,cin])

Key structure exploited:
  - The W-axis sample coords are compile-time constants -> plane reduces to
    B[c, i, cin] (128 x 6 x 6) on the host.
  - H-axis interpolation weights are tent functions: val[c,n,cin] =
    sum_i tent_i(5*x[n,cin]) * B[c,i,cin].
  - Pair the 6 cins into 3 pairs: pv_p[c,n] = val(2p)*val(2p+1) =
    sum_{i,j} (B[c,i,2p]*B[c,j,2p+1]) * (tent_i(iy_2p)*tent_j(iy_2p+1))
    -> a K=36 matmul per pair with host-precomputed tables PB_p [36, 128]
    and per-ray weights pw_p [36, n] (tent products, computed on host).
  - feat = pv0*pv1*pv2 elementwise (VectorE), then z[n, o] = sum_c feat*G
    as a matmul with feat (bf16) as weights (K=C=128), then sigmoid.

Sharding: pure data-parallel over rays; 8 cores, each runs the same NEFF on
its 16384-ray shard. Host scatters pw and gathers y.
"""

import numpy as np
import ml_dtypes

N_COMP = 16
OUT_CH = 8
N_RAYS = 131072
IN_CH = 6
WIDTH = 512
C = N_COMP * OUT_CH  # 128

N_CORES = 8
N_PER_CORE = N_RAYS // N_CORES  # 16384
TILE = 512
N_TILES = N_PER_CORE // TILE  # 32

_CACHE = {}


def _build_nc():
    import concourse.bass as bass
    import concourse.mybir as mybir
    from concourse import bacc
    from concourse.tile import TileContext
    from concourse.bass import ts
    from contextlib import ExitStack

    f32 = mybir.dt.float32
    bf16 = mybir.dt.bfloat16

    nc = bacc.Bacc("TRN2", debug=False, num_devices=N_CORES)

    pw_d = nc.dram_tensor("pw", [36, 3, N_PER_CORE], bf16, kind="ExternalInput")
    pb_d = nc.dram_tensor("pb", [36, 3 * 128], bf16, kind="ExternalInput")
    g_d = nc.dram_tensor("g", [C, OUT_CH], bf16, kind="ExternalInput")
    y_d = nc.dram_tensor("y", [N_PER_CORE, OUT_CH], f32, kind="ExternalOutput")

    # [p, t, b, o] view of the output: tile t covers rays [512t, 512t+512),
    # stored as 4 blocks of 128 rays (partition dim p first for DMA pairing).
    y_v = y_d.ap().rearrange("(t b p) o -> p t b o", p=128, b=4)
    pw_ap = pw_d.ap()

    SUP = 4  # tiles per super-tile (one DMA in/out per super)
    N_SUP = N_TILES // SUP

    with ExitStack() as ctx:
        tc = ctx.enter_context(TileContext(nc))
        consts = ctx.enter_context(tc.tile_pool(name="consts", bufs=1))
        pwp = ctx.enter_context(tc.tile_pool(name="pwp", bufs=4))
        sb = ctx.enter_context(tc.tile_pool(name="sb", bufs=8))
        ps = ctx.enter_context(tc.tile_pool(name="ps", bufs=1, space="PSUM"))
        ps2 = ctx.enter_context(tc.tile_pool(name="ps2", bufs=2, space="PSUM"))

        pb_t = consts.tile([36, 3 * 128], bf16)
        nc.scalar.dma_start(pb_t[:], pb_d.ap())
        g_t = consts.tile([C, OUT_CH], bf16)
        nc.scalar.dma_start(g_t[:], g_d.ap())

        # Groups of tiles; within a group, sub-tiles are processed in merged
        # pairs (one [128, 1024] product op spanning 2 PSUM banks) to amortize
        # DVE per-op overhead. Small first/last groups shrink fill/drain.
        groups = [(0, 2), (2, 2), (4, 2)]
        t0 = 6
        while t0 < N_TILES - 4:
            g = min(SUP, N_TILES - 4 - t0)
            groups.append((t0, g))
            t0 += g
        groups += [(N_TILES - 4, 2), (N_TILES - 2, 2)]

        for g_start, g_sz in groups:
            pw_t = pwp.tile([36, 3, SUP * TILE], bf16, tag="pw")
            nc.sync.dma_start(
                pw_t[:, :, : g_sz * TILE],
                pw_ap[:, :, g_start * TILE : (g_start + g_sz) * TILE],
            )

            zt = ps2.tile([128, SUP * 4 * OUT_CH], f32, tag="zt")
            for m in range(g_sz // 2):
                pvs = []
                for p in range(3):
                    pv = ps.tile([128, 2, TILE], f32, tag=f"pv{p}")
                    for h in range(2):
                        nc.tensor.matmul(
                            pv[:, h, :],
                            pb_t[:, ts(p, 128)],
                            pw_t[:, p, ts(2 * m + h, TILE)],
                            start=True,
                            stop=True,
                        )
                    pvs.append(pv)

                c0 = sb.tile([128, 2 * TILE], f32, tag="c0")
                nc.scalar.copy(c0[:], pvs[0][:].rearrange("p a b -> p (a b)"))
                q = sb.tile([128, 2 * TILE], f32, tag="q")
                nc.vector.tensor_tensor(
                    q[:],
                    c0[:],
                    pvs[1][:].rearrange("p a b -> p (a b)"),
                    mybir.AluOpType.mult,
                )
                feat = sb.tile([128, 2 * TILE], bf16, tag="feat")
                nc.vector.tensor_tensor(
                    feat[:],
                    q[:],
                    pvs[2][:].rearrange("p a b -> p (a b)"),
                    mybir.AluOpType.mult,
                )
                for b in range(8):
                    nc.tensor.matmul(
                        zt[:, ts(m * 8 + b, OUT_CH)],
                        feat[:, ts(b, 128)],
                        g_t[:],
                        start=True,
                        stop=True,
                    )

            sig = sb.tile([128, SUP * 4 * OUT_CH], f32, tag="sig")
            nc.scalar.activation(
                sig[:, : g_sz * 4 * OUT_CH],
                zt[:, : g_sz * 4 * OUT_CH],
                mybir.ActivationFunctionType.Sigmoid,
            )
            nc.scalar.dma_start(
                y_v[:, g_start : g_start + g_sz],
                sig[:, : g_sz * 4 * OUT_CH].rearrange(
                    "p (t b o) -> p t b o", o=OUT_CH, b=4
                ),
            )
    nc.compile()
    return nc


def _host_tables(plane):
    """B[c,i,cin] from plane via the constant W-axis lerp; pair tables PB, selector G."""
    plane64 = plane.astype(np.float64)
    h_loc = np.linspace(-1.0, 1.0, IN_CH, dtype=np.float32)
    ix = (h_loc + np.float32(1.0)) * np.float32(0.5) * np.float32(WIDTH - 1)
    j0 = np.clip(np.floor(ix).astype(np.int32), 0, WIDTH - 1)
    j1 = np.clip(j0 + 1, 0, WIDTH - 1)
    wx = (ix - j0.astype(np.float32)).astype(np.float64)  # [6]

    # B[c, i, cin] = (1-wx[cin]) * plane[c, i, j0[cin]] + wx[cin] * plane[c, i, j1[cin]]
    B = (1.0 - wx)[None, None, :] * plane64[:, :, j0] + wx[None, None, :] * plane64[:, :, j1]

    # PB_p[(i,j), c] = B[c, i, 2p] * B[c, j, 2p+1]; layout [36, 3*128] bf16
    PB = np.empty((36, 3 * 128), dtype=np.float64)
    for p in range(3):
        prod = B[:, :, None, 2 * p] * B[:, None, :, 2 * p + 1]  # [c, i, j]
        PB[:, p * 128 : (p + 1) * 128] = prod.reshape(C, 36).T
    PBb = PB.astype(ml_dtypes.bfloat16)

    G = np.zeros((C, OUT_CH), dtype=ml_dtypes.bfloat16)
    for c in range(C):
        G[c, c % OUT_CH] = 1.0
    return PBb, G


def _host_pw(x):
    """Per-ray pair weights pw[(i,j), p, n] = tent_i(iy[n,2p]) * tent_j(iy[n,2p+1]), bf16."""
    x = np.asarray(x, dtype=np.float32)
    # Match reference fp32 arithmetic for iy.
    norm = x * np.float32(2.0) - np.float32(1.0)
    iy = (norm + np.float32(1.0)) * np.float32(0.5) * np.float32(IN_CH - 1)  # [N, 6]
    # Clamp to the grid so out-of-range coords reproduce the reference's
    # clip-to-edge behavior (tent_0(0)=1 / tent_5(5)=1). No-op for x in [0,1].
    iy = np.clip(iy, np.float32(0.0), np.float32(IN_CH - 1))
    k = np.arange(IN_CH, dtype=np.float32)
    T = np.maximum(np.float32(0.0), np.float32(1.0) - np.abs(iy[:, :, None] - k))  # [N, 6, 6]
    pw = np.empty((36, 3, N_RAYS), dtype=ml_dtypes.bfloat16)
    for p in range(3):
        prod = T[:, 2 * p, :, None] * T[:, 2 * p + 1, None, :]  # [N, i, j]
        pw[:, p, :] = prod.reshape(N_RAYS, 36).T.astype(ml_dtypes.bfloat16)
    return pw


def kernel(x, plane):
    from concourse.bass_utils import run_bass_kernel_spmd

    if "nc" not in _CACHE:
        _CACHE["nc"] = _build_nc()
    nc = _CACHE["nc"]

    PB, G = _host_tables(np.asarray(plane))
    pw = _host_pw(x)

    in_maps = []
    for i in range(N_CORES):
        s = i * N_PER_CORE
        in_maps.append(
            {
                "pw": np.ascontiguousarray(pw[:, :, s : s + N_PER_CORE]),
                "pb": PB,
                "g": G,
            }
        )
    res = run_bass_kernel_spmd(nc, in_maps, core_ids=list(range(N_CORES)))
    return np.concatenate([r["y"] for r in res.results], axis=0)
